# revision 50
# baseline (speedup 1.0000x reference)
"""Distributed attention kernel for Trainium2 (8 NeuronCores), v2.

Problem: B=2, L=2048, DIM=1024, H=16 heads, HD=64.
  qkv = x @ Wqkv; q,k = rmsnorm per head (+scales); RoPE(q, k);
  scores = q k^T / sqrt(HD); p = softmax(scores); o = p v;
  out = o @ Wproj + bproj.

Sharding: tensor-parallel over heads -- 2 heads per core.

v2 structure (vs v1): batch-major fused pipeline tuned for the PE HAM
clock gate and the ScalarE exp roofline:
  - ScalarE runs ONLY table ops from one set (natural_log_exp_and_others):
    softmax Exp plus the rmsnorm 1/sqrt as Exp(-0.5*Ln(ssq)). No table
    thrash, no ACT copies (those moved to DVE).
  - score matmuls for the two heads are emitted adjacently; K=64 lhsT at
    base partitions 0/64 auto-derives tile_position (0,0)/(64,0) so the
    PE runs them concurrently (row packing, 2x).
  - v transposed via DMA xbar (dma_start_transpose), not the PE.
  - attention sweeps are per (b, l-chunk) with po accumulators for both
    heads; PSUM budget: qkv/proj 2 + aux 2 + st 2 + po 2 = 8 banks.
  - qkv for b=1 is emitted after attention b=0 so the Tile scheduler
    fills PE idle slots (keeps HAM warm).
  - per-batch AllToAll (bf16) head-shard -> sequence-shard; proj after
    both A2As; A2A(b0) overlaps attention b=1.
Host concatenates 8 x [1024, 512] column shards (256 cols per b) and
transposes.
"""

import sys

if "/opt/trn_rl_repo" not in sys.path:
    sys.path.insert(0, "/opt/trn_rl_repo")

import numpy as np
import ml_dtypes

B, L, DIM, H, HD = 2, 2048, 1024, 16, 64
NC = 8
HPC = H // NC          # heads per core = 2
BL = B * L             # 4096 flattened rows
CH = 512               # l-chunk size
NCH = BL // CH         # 8 chunks
EPS = 1e-6
THETA = 10000.0
F = 3 * HPC * HD       # 384 qkv features per core

BF = ml_dtypes.bfloat16
_CACHE = {}


def _rope_tables():
    inv_freq = 1.0 / (THETA ** (np.arange(0, HD, 2, dtype=np.float64) / HD))  # [32]
    ang = np.arange(L, dtype=np.float64)[None, :] * inv_freq[:, None]  # [32, L]
    return np.cos(ang), np.sin(ang)


def _make_tables(scale, fold):
    """[128, L] bf16 cos/sin tables (rows duplicated for the two heads),
    per-feature scale folded in.

    Device computes on the full 128-row (2-head) qkv tile, per head at
    row offset r0 in {0, 64}:
      tc = src * ct                          (one [128] mul)
      ts[r0+ 0:r0+32] = src[r0+32:r0+64] * st[r0+32:r0+64]
      ts[r0+32:r0+64] = src[r0+ 0:r0+32] * st[r0+ 0:r0+32]
      out = tc + ts
    which equals rotate-half RoPE with scale/fold applied (sin sign for
    the first half is folded into st rows 32:64). Row duplication keeps
    every DVE tensor_tensor partition-aligned.
    """
    c, s = _rope_tables()
    ct = np.empty((HD, L), np.float64)
    st = np.empty((HD, L), np.float64)
    ct[0:32] = c * (scale[0:32, None] * fold)
    ct[32:64] = c * (scale[32:64, None] * fold)
    st[0:32] = s * (scale[0:32, None] * fold)
    st[32:64] = -s * (scale[32:64, None] * fold)
    ct2 = np.concatenate([ct, ct], axis=0)
    st2 = np.concatenate([st, st], axis=0)
    return ct2.astype(BF), st2.astype(BF)


def _host_inputs(x, Wqkv, q_scale, k_scale, Wproj, bproj):
    x2 = np.ascontiguousarray(np.asarray(x, np.float32).reshape(BL, DIM))
    xT = np.ascontiguousarray(x2.T.astype(BF))              # [DIM, BL] bf16
    Wqkv = np.asarray(Wqkv, np.float32)
    Wq = Wqkv[:, 0 * DIM:1 * DIM].reshape(DIM, H, HD)
    Wk = Wqkv[:, 1 * DIM:2 * DIM].reshape(DIM, H, HD)
    Wv = Wqkv[:, 2 * DIM:3 * DIM].reshape(DIM, H, HD)

    qc, qs = _make_tables(np.asarray(q_scale, np.float64), 1.0 / np.sqrt(HD))
    kc, ks = _make_tables(np.asarray(k_scale, np.float64), 1.0)

    ind2col = np.zeros((128, 2), BF)
    ind2col[0:64, 0] = 1.0
    ind2col[64:128, 1] = 1.0
    # 8.0 = sqrt(HD): folds the /HD of the mean-square into the
    # broadcast so the device computes 8/sqrt(ssq) = 1/sqrt(ssq/64).
    indbc = np.zeros((2, 128), BF)
    indbc[0, 0:64] = 8.0
    indbc[1, 64:128] = 8.0
    ones64 = np.ones((1, 64), BF)
    ident = np.eye(128, dtype=BF)
    wp = np.ascontiguousarray(np.asarray(Wproj, np.float32).astype(BF))
    bp = np.ascontiguousarray(
        np.asarray(bproj, np.float32).reshape(8, 128).T)    # [128, 8]

    shared = dict(xT=xT, qc=qc, qs=qs, kc=kc, ks=ks, ind2col=ind2col,
                  indbc=indbc, ones64=ones64, ident=ident, wp=wp, bp=bp)
    in_maps = []
    for c in range(NC):
        hA, hB = HPC * c, HPC * c + 1
        wqc = np.concatenate(
            [Wq[:, hA], Wq[:, hB], Wk[:, hA], Wk[:, hB], Wv[:, hA], Wv[:, hB]],
            axis=1)                                        # [DIM, 384]
        m = dict(shared)
        m["wq"] = np.ascontiguousarray(wqc.astype(BF))
        in_maps.append(m)
    return in_maps


def _patch_act_tables():
    """Steer the ACT table-load pass so Exp/Ln/Copy all resolve to the one
    set that contains all three (natural_log_exp_and_others). The default
    per-function assignment picks exp_and_others for Exp and
    natural_log_exp_and_others for Ln, reloading tables (~2.7us) on every
    rmsnorm <-> softmax transition."""
    import concourse.mybir as mybir
    from concourse import hw_specs, bacc

    if getattr(bacc, "_act_tables_patched", False):
        return
    AF = mybir.ActivationFunctionType
    orig = hw_specs.get_activation_tables

    def patched(arch):
        tables = orig(arch)
        keep = {"natural_log_exp_and_others"}
        for name, fns in tables.items():
            if name not in keep:
                fns.discard(AF.Exp)
                fns.discard(AF.Ln)
                fns.discard(AF.Copy)
        return tables

    bacc.get_activation_tables = patched
    bacc._act_tables_patched = True


def _build(taps=False):
    import concourse.bass as bass  # noqa: F401
    import concourse.mybir as mybir
    import concourse.tile as tile
    from concourse import bacc

    _patch_act_tables()

    fp32 = mybir.dt.float32
    bf16 = mybir.dt.bfloat16
    AF = mybir.ActivationFunctionType

    nc = bacc.Bacc("TRN2", target_bir_lowering=False, debug=False,
                   num_devices=NC)

    xT = nc.dram_tensor("xT", [DIM, BL], bf16, kind="ExternalInput")
    wq = nc.dram_tensor("wq", [DIM, F], bf16, kind="ExternalInput")
    qc = nc.dram_tensor("qc", [128, L], bf16, kind="ExternalInput")
    qs = nc.dram_tensor("qs", [128, L], bf16, kind="ExternalInput")
    kc = nc.dram_tensor("kc", [128, L], bf16, kind="ExternalInput")
    ks = nc.dram_tensor("ks", [128, L], bf16, kind="ExternalInput")
    ind2col_d = nc.dram_tensor("ind2col", [128, 2], bf16,
                               kind="ExternalInput")
    indbc_d = nc.dram_tensor("indbc", [2, 128], bf16, kind="ExternalInput")
    ones64_d = nc.dram_tensor("ones64", [1, 64], bf16, kind="ExternalInput")
    ident_d = nc.dram_tensor("ident", [128, 128], bf16, kind="ExternalInput")
    wp_d = nc.dram_tensor("wp", [DIM, DIM], bf16, kind="ExternalInput")
    bp_d = nc.dram_tensor("bp", [128, 8], fp32, kind="ExternalInput")
    out_d = nc.dram_tensor("out", [DIM, 2 * 256], fp32, kind="ExternalOutput")
    if taps:
        tap_qtn = nc.dram_tensor("tap_qtn", [128, CH], bf16,
                                 kind="ExternalOutput")
        tap_ktn = nc.dram_tensor("tap_ktn", [128, CH], bf16,
                                 kind="ExternalOutput")
        tap_v = nc.dram_tensor("tap_v", [128, 4 * 130], bf16,
                               kind="ExternalOutput")
        tap_a2ain = nc.dram_tensor("tap_a2ain", [NC * 128, 256], bf16,
                                   kind="ExternalOutput")
        tap_a2aout = nc.dram_tensor("tap_a2aout", [NC * 128, 256], bf16,
                                    kind="ExternalOutput")

    from contextlib import ExitStack

    with tile.TileContext(nc) as tc:
        with ExitStack() as stack:
            ep = stack.enter_context
            consts = ep(tc.tile_pool(name="consts", bufs=1))
            wqp = ep(tc.tile_pool(name="wqp", bufs=1))
            tabs = ep(tc.tile_pool(name="tabs", bufs=1))
            qkv_sb = ep(tc.tile_pool(name="qkv_sb", bufs=1))
            wppool = ep(tc.tile_pool(name="wppool", bufs=1))
            dram = ep(tc.tile_pool(name="dram", bufs=1, space="DRAM"))
            # streaming SBUF pools
            xtp = ep(tc.tile_pool(name="xt", bufs=20))
            stgp = ep(tc.tile_pool(name="stg", bufs=4))
            tmpp = ep(tc.tile_pool(name="tmp", bufs=8))
            smlp = ep(tc.tile_pool(name="sml", bufs=8))
            vtp = ep(tc.tile_pool(name="vt", bufs=2))
            ptp = ep(tc.tile_pool(name="pt", bufs=8))
            otp = ep(tc.tile_pool(name="ot", bufs=4))
            rdvp = ep(tc.tile_pool(name="rdv", bufs=4))
            ofp = ep(tc.tile_pool(name="ofp", bufs=8))
            obp = ep(tc.tile_pool(name="obp", bufs=2))
            # PSUM banks: qp 1 + aux 1 + st 2x2 + po 2 = 8
            qpp = ep(tc.tile_pool(name="qp", bufs=1, space="PSUM"))
            auxp = ep(tc.tile_pool(name="aux", bufs=1, space="PSUM"))
            stp = ep(tc.tile_pool(name="stp", bufs=2, space="PSUM"))
            pop = ep(tc.tile_pool(name="pop", bufs=2, space="PSUM"))

            ind2col = consts.tile([128, 2], bf16)
            nc.sync.dma_start(ind2col[:], ind2col_d[:])
            indbc = consts.tile([2, 128], bf16)
            nc.sync.dma_start(indbc[:], indbc_d[:])
            ones64 = consts.tile([1, 64], bf16)
            nc.sync.dma_start(ones64[:], ones64_d[:])
            ident = consts.tile([128, 128], bf16)
            nc.sync.dma_start(ident[:], ident_d[:])
            bp_sb = consts.tile([128, 8], fp32)
            nc.sync.dma_start(bp_sb[:], bp_d[:])

            # tables are loaded inside the emission section, after the
            # first x chunk's DMAs (they are only needed once rope starts)
            qc_sb = tabs.tile([128, L], bf16)
            qs_sb = tabs.tile([128, L], bf16)
            kc_sb = tabs.tile([128, L], bf16)
            ks_sb = tabs.tile([128, L], bf16)

            def load_tables():
                nc.sync.dma_start(qc_sb[:], qc[:])
                nc.sync.dma_start(qs_sb[:], qs[:])
                nc.sync.dma_start(kc_sb[:], kc[:])
                nc.sync.dma_start(ks_sb[:], ks[:])

            wq_sb = []
            for kk in range(8):
                t = wqp.tile([128, F], bf16, name=f"wq{kk}")
                nc.sync.dma_start(t[:], wq[128 * kk:128 * (kk + 1), :])
                wq_sb.append(t)
            wp_sb = [wppool.tile([128, DIM], bf16, name=f"wp{ff}")
                     for ff in range(8)]

            def load_wp():
                for ff in range(8):
                    nc.sync.dma_start(wp_sb[ff][:],
                                      wp_d[128 * ff:128 * (ff + 1), :])

            # per-chunk persistent activations
            qTn = [qkv_sb.tile([128, CH], bf16, name=f"qTn{c}")
                   for c in range(NCH)]
            kTn = [qkv_sb.tile([128, CH], bf16, name=f"kTn{c}")
                   for c in range(NCH)]
            # v per chunk: 4 m-tiles of [128, 130] (64 vA | 1 | 64 vB | 1)
            v_sb = [qkv_sb.tile([128, 4 * 130], bf16, name=f"v{c}")
                    for c in range(NCH)]
            for c in range(NCH):
                nc.gpsimd.memset(v_sb[c][:], 1.0)

            # per-batch A2A buffers: dest core j's block is this core's two
            # heads (128 rows) for j's 256 columns of batch b
            a2a_in = [dram.tile([NC * 128, 256], bf16, name=f"a2a_in{b}")
                      for b in range(2)]
            a2a_out = [dram.tile([NC * 128, 256], bf16, name=f"a2a_out{b}")
                       for b in range(2)]

            def qkv_piece(ch, xt, tix):
                """One tix (q=0 / k=1 / v=2) of a chunk's qkv+rope; ~1.7us
                of dense PE work so it can interleave with attention."""
                lsl = slice(CH * (ch % 4), CH * (ch % 4) + CH)
                ct, stb, dst = [(qc_sb, qs_sb, qTn[ch]),
                                (kc_sb, ks_sb, kTn[ch]),
                                (None, None, None)][tix]
                p = qpp.tile([128, CH], fp32, tag="qp")
                for kk in range(8):
                    nc.tensor.matmul(
                        p[:], wq_sb[kk][:, 128 * tix:128 * (tix + 1)],
                        xt[kk][:], start=(kk == 0), stop=(kk == 7))
                if tix == 2:
                    # v: bf16 copy now; the PE transpose is a separate
                    # piece (v_finish) so it never FIFO-stalls the PE
                    # waiting on this DVE copy
                    vt = vtp.tile([128, CH], bf16, tag="vt",
                                  name=f"vt{ch}")
                    nc.vector.tensor_copy(vt[:], p[:])
                    vt_hold[ch] = vt
                    return
                # rmsnorm: ssq -> 1/sqrt via Exp(-0.5*Ln) -> broadcast
                stgd = stgp.tile([128, CH], bf16, tag="stg")
                nc.vector.tensor_copy(stgd[:], p[:])
                sq = stgp.tile([128, CH], bf16, tag="sq")
                nc.vector.tensor_mul(sq[:], stgd[:], stgd[:])
                ssq = auxp.tile([2, CH], fp32, tag="aux")
                nc.tensor.matmul(ssq[:], ind2col[:], sq[:],
                                 start=True, stop=True)
                lssq = smlp.tile([2, CH], fp32, tag="lssq")
                nc.scalar.activation(lssq[:], ssq[:], AF.Ln)
                ivb = smlp.tile([2, CH], bf16, tag="ivb")
                nc.scalar.activation(ivb[:], lssq[:], AF.Exp, scale=-0.5)
                invb = auxp.tile([128, CH], fp32, tag="aux")
                nc.tensor.matmul(invb[:], indbc[:], ivb[:],
                                 start=True, stop=True)
                invbs = tmpp.tile([128, CH], bf16, tag="invbs")
                nc.vector.tensor_copy(invbs[:], invb[:])
                # rope on the bf16 copy (tables are head-duplicated
                # [128, L] so every mul is partition-aligned)
                tc_ = tmpp.tile([128, CH], bf16, tag="tc")
                nc.vector.tensor_mul(tc_[:], stgd[:], ct[:, lsl])
                ts_ = tmpp.tile([128, CH], bf16, tag="ts")
                for h in range(2):
                    r0 = 64 * h
                    nc.vector.tensor_mul(
                        ts_[r0:r0 + 32, :], stgd[r0 + 32:r0 + 64, :],
                        stb[r0 + 32:r0 + 64, lsl])
                    nc.vector.tensor_mul(
                        ts_[r0 + 32:r0 + 64, :], stgd[r0:r0 + 32, :],
                        stb[r0:r0 + 32, lsl])
                o12 = tmpp.tile([128, CH], bf16, tag="o12")
                nc.vector.tensor_add(o12[:], tc_[:], ts_[:])
                nc.vector.tensor_mul(dst[:, :], o12[:], invbs[:])

            vt_hold = {}

            def v_finish(ch):
                # PE transpose of v, interleaved next to the ones columns
                # ([64 vA | 1 | 64 vB | 1] per m-tile)
                vt = vt_hold.pop(ch)
                tp = qpp.tile([128, CH], bf16, tag="qp")
                for blk in range(4):
                    nc.tensor.transpose(
                        tp[:, 128 * blk:128 * (blk + 1)],
                        vt[:, 128 * blk:128 * (blk + 1)], ident[:])
                for blk in range(4):
                    nc.vector.tensor_copy(
                        v_sb[ch][:, 130 * blk:130 * blk + 64],
                        tp[:, 128 * blk:128 * blk + 64])
                    nc.vector.tensor_copy(
                        v_sb[ch][:, 130 * blk + 65:130 * blk + 129],
                        tp[:, 128 * blk + 64:128 * (blk + 1)])

            def load_xt(ch):
                c0 = CH * ch
                xt = []
                for kk in range(8):
                    t = xtp.tile([128, CH], bf16, tag="xt")
                    nc.sync.dma_start(
                        t[:], xT[128 * kk:128 * (kk + 1), c0:c0 + CH])
                    xt.append(t)
                return xt

            def proj_piece(b, dd, of):
                pr = qpp.tile([128, 256], fp32, tag="qp")
                for ff in range(8):
                    nc.tensor.matmul(
                        pr[:], wp_sb[ff][:, 128 * dd:128 * (dd + 1)],
                        of[ff][:], start=(ff == 0), stop=(ff == 7))
                ob = obp.tile([128, 256], fp32, tag="ob")
                nc.vector.tensor_scalar_add(ob[:], pr[:],
                                            bp_sb[:, dd:dd + 1])
                nc.sync.dma_start(
                    out_d[128 * dd:128 * (dd + 1), 256 * b:256 * b + 256],
                    ob[:])

            def load_of(b):
                of = []
                for ff in range(8):
                    t = ofp.tile([128, 256], bf16, tag="of",
                                 name=f"of{b}_{ff}")
                    nc.sync.dma_start(
                        t[:], a2a_out[b][128 * ff:128 * (ff + 1), :])
                    of.append(t)
                return of

            def attention_batch(b, filler):
                """32 super-iterations (4 sweeps x 8 m-pairs), software-
                pipelined: each super-iter computes scores for two adjacent
                m-tiles per head into a [128, 1024] two-bank PSUM tile and
                runs one FD=1024 exp per head (amortizes the ACT per-
                instruction overhead). o-matmuls lag by one super-iter so
                the in-order PE queue never stalls on a fresh exp; the
                softmax division trails each sweep by 1-3 super-iters.
                `filler` maps super-iter -> thunks emitting independent PE
                work (next batch's qkv, previous batch's proj)."""
                pts_hist = {}
                po_all = {}
                div_q = {}

                def emit_o(u):
                    s, mp = u // 8, 2 * (u % 8)
                    pts = pts_hist.pop(u)
                    for dm in range(2):
                        m = mp + dm
                        cm = 4 * b + m // 4
                        vo = 130 * (m % 4)
                        for h in range(2):
                            nc.tensor.matmul(
                                po_all[s][h][:],
                                v_sb[cm][:, vo + 65 * h:vo + 65 * h + 65],
                                pts[h][:, CH * dm:CH * (dm + 1)],
                                start=(m == 0), stop=(m == 15))

                def div_stage(s, stage):
                    # stage 0: denominators -> reciprocal; 1: broadcast;
                    # 2: divide + stage into the A2A buffer
                    st_ = div_q[s]
                    if stage == 0:
                        po = po_all[s]
                        for h in range(2):
                            rc0 = rdvp.tile([1, CH], fp32, tag="rc0")
                            nc.scalar.activation(rc0[:], po[h][64:65, :],
                                                 AF.Copy)
                            rc = rdvp.tile([1, CH], fp32, tag="rc")
                            nc.vector.reciprocal_approx_fast(rc[:], rc0[:])
                            rcb = rdvp.tile([1, CH], bf16, tag="rcb")
                            nc.vector.tensor_copy(rcb[:], rc[:])
                            st_.append(rcb)
                    elif stage == 1:
                        for h in range(2):
                            rb = stp.tile([64, CH], fp32, tag="st")
                            nc.tensor.matmul(rb[:], ones64[:], st_[h][:],
                                             start=True, stop=True)
                            rbs = rdvp.tile([64, CH], bf16, tag="rbs")
                            nc.vector.tensor_copy(rbs[:], rb[:])
                            st_.append(rbs)
                    else:
                        po = po_all.pop(s)
                        for h in range(2):
                            ot = otp.tile([64, CH], bf16, tag="ot")
                            nc.vector.tensor_mul(ot[:], po[h][0:64, :],
                                                 st_[2 + h][:])
                            for half in range(2):
                                j = 2 * s + half
                                nc.sync.dma_start(
                                    a2a_in[b][128 * j + 64 * h:
                                              128 * j + 64 * h + 64, :],
                                    ot[:, 256 * half:256 * half + 256])
                        del div_q[s]

                for u in range(36):
                    # scores first: hands ACT its next exp as early as
                    # possible (ACT throughput paces the attention region)
                    if u < 32:
                        s, mp = u // 8, 2 * (u % 8)
                        if mp == 0:
                            po_all[s] = [
                                pop.tile([65, CH], fp32, tag="po",
                                         name=f"po{b}{s}{h}")
                                for h in range(2)]
                        cl = 4 * b + s
                        sts = [stp.tile([128, 2 * CH], fp32, tag="st",
                                        name=f"st{b}_{u}_{h}")
                               for h in range(2)]
                        # dm outer / h inner keeps the two heads' K=64
                        # matmuls adjacent -> concurrent row-tiled pairs
                        for dm in range(2):
                            m = mp + dm
                            cm = 4 * b + m // 4
                            mo = 128 * (m % 4)
                            for h in range(2):
                                hr = slice(64 * h, 64 * h + 64)
                                nc.tensor.matmul(
                                    sts[h][:, CH * dm:CH * (dm + 1)],
                                    kTn[cm][hr, mo:mo + 128],
                                    qTn[cl][hr, :], start=True, stop=True)
                        pts = []
                        for h in range(2):
                            pt = ptp.tile([128, 2 * CH], bf16, tag="pt")
                            nc.scalar.activation(pt[:], sts[h][:], AF.Exp)
                            pts.append(pt)
                        pts_hist[u] = pts
                    if 1 <= u <= 32:
                        emit_o(u - 1)
                    # division stages trail each sweep by 1/2/3 super-iters
                    # so nothing ever waits on a fresh dependency
                    if u >= 9 and (u - 9) % 8 == 0 and (u - 9) // 8 < 4:
                        s_ = (u - 9) // 8
                        div_q[s_] = []
                        div_stage(s_, 0)
                    if u >= 10 and (u - 10) % 8 == 0 and (u - 10) // 8 < 4:
                        div_stage((u - 10) // 8, 1)
                    if u >= 11 and (u - 11) % 8 == 0 and (u - 11) // 8 < 4:
                        div_stage((u - 11) // 8, 2)
                    for thunk in filler.get(u, []):
                        thunk()

            import concourse.mybir as mybir2

            def emit_a2a(b):
                nc.gpsimd.collective_compute(
                    "AllToAll", mybir2.AluOpType.bypass,
                    replica_groups=[list(range(NC))],
                    ins=[a2a_in[b][:]],
                    outs=[a2a_out[b][:]],
                )

            # ---- emission ----
            # prefix: qkv for b=0 (attention b0 starts as chunk 0 lands);
            # x chunk 0 is the critical first DMA, tables follow it
            for ch in range(4):
                xt = load_xt(ch)
                if ch == 0:
                    load_tables()
                for tix in range(3):
                    qkv_piece(ch, xt, tix)
                v_finish(ch)
            load_wp()  # needed only by proj; keep off the startup DMA path

            # attention b0 with b1's qkv interleaved as PE filler; the
            # last chunk's pieces land on the sweep-3 division drain so
            # the PE never idles long enough for HAM to re-throttle
            # qkv pieces sit on the sweep-boundary division windows
            # (u = 8s+9..), where attention itself leaves the PE sparse
            filler0 = {}
            xt_hold = {}
            for i, ch in enumerate(range(4, 8)):
                base = (4, 12, 20, 28)[i]

                def mk_load(ch=ch):
                    xt_hold[ch] = load_xt(ch)
                filler0.setdefault(base - 1, []).append(mk_load)
                for tix in range(3):
                    def mk(ch=ch, tix=tix):
                        qkv_piece(ch, xt_hold[ch], tix)
                    filler0.setdefault(base + 2 * tix, []).append(mk)

                def mkv(ch=ch):
                    v_finish(ch)
                filler0.setdefault(base + 6, []).append(mkv)
            attention_batch(0, filler0)
            emit_a2a(0)

            # attention b1 with proj(b0) on its division windows
            filler1 = {}
            of_hold = {}

            def mk_of0():
                of_hold[0] = load_of(0)
            filler1.setdefault(5, []).append(mk_of0)
            for dd, pos in enumerate((9, 10, 17, 18, 25, 26, 33, 34)):
                def mkp(dd=dd):
                    proj_piece(0, dd, of_hold[0])
                filler1.setdefault(pos, []).append(mkp)
            attention_batch(1, filler1)
            emit_a2a(1)

            of1 = load_of(1)
            for dd in range(8):
                proj_piece(1, dd, of1)

            if taps:
                nc.sync.dma_start(tap_qtn[:], qTn[0][:])
                nc.sync.dma_start(tap_ktn[:], kTn[0][:])
                nc.sync.dma_start(tap_v[:], v_sb[0][:])
                nc.sync.dma_start(tap_a2ain[:], a2a_in[0][:])
                nc.sync.dma_start(tap_a2aout[:], a2a_out[0][:])

    nc.compile()
    return nc


def _run(inputs, trace=False, trace_kwargs=None):
    from concourse.bass_utils import run_bass_kernel_spmd

    if "nc" not in _CACHE:
        _CACHE["nc"] = _build()
    nc = _CACHE["nc"]
    in_maps = _host_inputs(**inputs)
    res = run_bass_kernel_spmd(
        nc, in_maps, core_ids=list(range(NC)), trace=trace,
        **(trace_kwargs or {}))
    return res


def _gather(res):
    fullT = np.empty((DIM, BL), np.float32)
    for c in range(NC):
        t = res.results[c]["out"]
        fullT[:, 256 * c:256 * (c + 1)] = t[:, 0:256]
        fullT[:, 2048 + 256 * c:2048 + 256 * (c + 1)] = t[:, 256:512]
    return np.ascontiguousarray(fullT.T).reshape(B, L, DIM).astype(np.float32)


def kernel(x, Wqkv, q_scale, k_scale, Wproj, bproj):
    res = _run(dict(x=x, Wqkv=Wqkv, q_scale=q_scale, k_scale=k_scale,
                    Wproj=Wproj, bproj=bproj))
    return _gather(res)


if __name__ == "__main__":
    rng = np.random.default_rng(0)
    x = rng.standard_normal((B, L, DIM), dtype=np.float32)
    Wqkv_ = rng.standard_normal((DIM, 3 * DIM), dtype=np.float32) * DIM ** -0.5
    Wproj_ = rng.standard_normal((DIM, DIM), dtype=np.float32) * DIM ** -0.5
    out = kernel(x=x, Wqkv=Wqkv_, q_scale=np.ones(HD, np.float32),
                 k_scale=np.ones(HD, np.float32), Wproj=Wproj_,
                 bproj=np.zeros(DIM, np.float32))
    print(out.shape, out.dtype)


# revision 51
# speedup vs baseline: 1.2593x; 1.2593x over previous
"""Distributed attention kernel for Trainium2 (8 NeuronCores), v2.

Problem: B=2, L=2048, DIM=1024, H=16 heads, HD=64.
  qkv = x @ Wqkv; q,k = rmsnorm per head (+scales); RoPE(q, k);
  scores = q k^T / sqrt(HD); p = softmax(scores); o = p v;
  out = o @ Wproj + bproj.

Sharding: tensor-parallel over heads -- 2 heads per core.

v2 structure (vs v1): batch-major fused pipeline tuned for the PE HAM
clock gate and the ScalarE exp roofline:
  - ScalarE runs ONLY table ops from one set (natural_log_exp_and_others):
    softmax Exp plus the rmsnorm 1/sqrt as Exp(-0.5*Ln(ssq)). No table
    thrash, no ACT copies (those moved to DVE).
  - score matmuls for the two heads are emitted adjacently; K=64 lhsT at
    base partitions 0/64 auto-derives tile_position (0,0)/(64,0) so the
    PE runs them concurrently (row packing, 2x).
  - v transposed via DMA xbar (dma_start_transpose), not the PE.
  - attention sweeps are per (b, l-chunk) with po accumulators for both
    heads; PSUM budget: qkv/proj 2 + aux 2 + st 2 + po 2 = 8 banks.
  - qkv for b=1 is emitted after attention b=0 so the Tile scheduler
    fills PE idle slots (keeps HAM warm).
  - per-batch AllToAll (bf16) head-shard -> sequence-shard; proj after
    both A2As; A2A(b0) overlaps attention b=1.
Host concatenates 8 x [1024, 512] column shards (256 cols per b) and
transposes.
"""

import sys

if "/opt/trn_rl_repo" not in sys.path:
    sys.path.insert(0, "/opt/trn_rl_repo")

import numpy as np
import ml_dtypes

B, L, DIM, H, HD = 2, 2048, 1024, 16, 64
NC = 8
HPC = H // NC          # heads per core = 2
BL = B * L             # 4096 flattened rows
CH = 512               # l-chunk size
NCH = BL // CH         # 8 chunks
EPS = 1e-6
THETA = 10000.0
F = 3 * HPC * HD       # 384 qkv features per core

BF = ml_dtypes.bfloat16
_CACHE = {}


def _rope_tables():
    inv_freq = 1.0 / (THETA ** (np.arange(0, HD, 2, dtype=np.float64) / HD))  # [32]
    ang = np.arange(L, dtype=np.float64)[None, :] * inv_freq[:, None]  # [32, L]
    return np.cos(ang), np.sin(ang)


def _make_tables(scale, fold):
    """[128, L] bf16 cos/sin tables (rows duplicated for the two heads),
    per-feature scale folded in.

    Device computes on the full 128-row (2-head) qkv tile, per head at
    row offset r0 in {0, 64}:
      tc = src * ct                          (one [128] mul)
      ts[r0+ 0:r0+32] = src[r0+32:r0+64] * st[r0+32:r0+64]
      ts[r0+32:r0+64] = src[r0+ 0:r0+32] * st[r0+ 0:r0+32]
      out = tc + ts
    which equals rotate-half RoPE with scale/fold applied (sin sign for
    the first half is folded into st rows 32:64). Row duplication keeps
    every DVE tensor_tensor partition-aligned.
    """
    c, s = _rope_tables()
    ct = np.empty((HD, L), np.float64)
    st = np.empty((HD, L), np.float64)
    ct[0:32] = c * (scale[0:32, None] * fold)
    ct[32:64] = c * (scale[32:64, None] * fold)
    st[0:32] = s * (scale[0:32, None] * fold)
    st[32:64] = -s * (scale[32:64, None] * fold)
    ct2 = np.concatenate([ct, ct], axis=0)
    st2 = np.concatenate([st, st], axis=0)
    return ct2.astype(BF), st2.astype(BF)


def _host_inputs(x, Wqkv, q_scale, k_scale, Wproj, bproj):
    x2 = np.ascontiguousarray(np.asarray(x, np.float32).reshape(BL, DIM))
    xT = np.ascontiguousarray(x2.T.astype(BF))              # [DIM, BL] bf16
    Wqkv = np.asarray(Wqkv, np.float32)
    Wq = Wqkv[:, 0 * DIM:1 * DIM].reshape(DIM, H, HD)
    Wk = Wqkv[:, 1 * DIM:2 * DIM].reshape(DIM, H, HD)
    Wv = Wqkv[:, 2 * DIM:3 * DIM].reshape(DIM, H, HD)

    qc, qs = _make_tables(np.asarray(q_scale, np.float64), 1.0 / np.sqrt(HD))
    kc, ks = _make_tables(np.asarray(k_scale, np.float64), 1.0)

    ind2col = np.zeros((128, 2), BF)
    ind2col[0:64, 0] = 1.0
    ind2col[64:128, 1] = 1.0
    # 8.0 = sqrt(HD): folds the /HD of the mean-square into the
    # broadcast so the device computes 8/sqrt(ssq) = 1/sqrt(ssq/64).
    indbc = np.zeros((2, 128), BF)
    indbc[0, 0:64] = 8.0
    indbc[1, 64:128] = 8.0
    ones64 = np.ones((1, 64), BF)
    ident = np.eye(128, dtype=BF)
    wp = np.ascontiguousarray(np.asarray(Wproj, np.float32).astype(BF))
    bp = np.ascontiguousarray(
        np.asarray(bproj, np.float32).reshape(8, 128).T)    # [128, 8]

    shared = dict(xT=xT, qc=qc, qs=qs, kc=kc, ks=ks, ind2col=ind2col,
                  indbc=indbc, ones64=ones64, ident=ident, wp=wp, bp=bp)
    in_maps = []
    for c in range(NC):
        hA, hB = HPC * c, HPC * c + 1
        wqc = np.concatenate(
            [Wq[:, hA], Wq[:, hB], Wk[:, hA], Wk[:, hB], Wv[:, hA], Wv[:, hB]],
            axis=1)                                        # [DIM, 384]
        m = dict(shared)
        m["wq"] = np.ascontiguousarray(wqc.astype(BF))
        in_maps.append(m)
    return in_maps


def _patch_act_tables():
    """Steer the ACT table-load pass so Exp/Ln/Copy all resolve to the one
    set that contains all three (natural_log_exp_and_others). The default
    per-function assignment picks exp_and_others for Exp and
    natural_log_exp_and_others for Ln, reloading tables (~2.7us) on every
    rmsnorm <-> softmax transition."""
    import concourse.mybir as mybir
    from concourse import hw_specs, bacc

    if getattr(bacc, "_act_tables_patched", False):
        return
    AF = mybir.ActivationFunctionType
    orig = hw_specs.get_activation_tables

    def patched(arch):
        tables = orig(arch)
        keep = {"natural_log_exp_and_others"}
        for name, fns in tables.items():
            if name not in keep:
                fns.discard(AF.Exp)
                fns.discard(AF.Ln)
                fns.discard(AF.Copy)
        return tables

    bacc.get_activation_tables = patched
    bacc._act_tables_patched = True


def _build(taps=False):
    import concourse.bass as bass  # noqa: F401
    import concourse.mybir as mybir
    import concourse.tile as tile
    from concourse import bacc

    _patch_act_tables()

    fp32 = mybir.dt.float32
    bf16 = mybir.dt.bfloat16
    AF = mybir.ActivationFunctionType

    nc = bacc.Bacc("TRN2", target_bir_lowering=False, debug=False,
                   num_devices=NC)

    xT = nc.dram_tensor("xT", [DIM, BL], bf16, kind="ExternalInput")
    wq = nc.dram_tensor("wq", [DIM, F], bf16, kind="ExternalInput")
    qc = nc.dram_tensor("qc", [128, L], bf16, kind="ExternalInput")
    qs = nc.dram_tensor("qs", [128, L], bf16, kind="ExternalInput")
    kc = nc.dram_tensor("kc", [128, L], bf16, kind="ExternalInput")
    ks = nc.dram_tensor("ks", [128, L], bf16, kind="ExternalInput")
    ind2col_d = nc.dram_tensor("ind2col", [128, 2], bf16,
                               kind="ExternalInput")
    indbc_d = nc.dram_tensor("indbc", [2, 128], bf16, kind="ExternalInput")
    ones64_d = nc.dram_tensor("ones64", [1, 64], bf16, kind="ExternalInput")
    ident_d = nc.dram_tensor("ident", [128, 128], bf16, kind="ExternalInput")
    wp_d = nc.dram_tensor("wp", [DIM, DIM], bf16, kind="ExternalInput")
    bp_d = nc.dram_tensor("bp", [128, 8], fp32, kind="ExternalInput")
    out_d = nc.dram_tensor("out", [DIM, 2 * 256], fp32, kind="ExternalOutput")
    if taps:
        tap_qtn = nc.dram_tensor("tap_qtn", [128, CH], bf16,
                                 kind="ExternalOutput")
        tap_ktn = nc.dram_tensor("tap_ktn", [128, CH], bf16,
                                 kind="ExternalOutput")
        tap_v = nc.dram_tensor("tap_v", [128, 4 * 130], bf16,
                               kind="ExternalOutput")
        tap_a2ain = nc.dram_tensor("tap_a2ain", [NC * 128, 256], bf16,
                                   kind="ExternalOutput")
        tap_a2aout = nc.dram_tensor("tap_a2aout", [NC * 128, 256], bf16,
                                    kind="ExternalOutput")

    from contextlib import ExitStack

    with tile.TileContext(nc) as tc:
        with ExitStack() as stack:
            ep = stack.enter_context
            consts = ep(tc.tile_pool(name="consts", bufs=1))
            wqp = ep(tc.tile_pool(name="wqp", bufs=1))
            tabs = ep(tc.tile_pool(name="tabs", bufs=1))
            qkv_sb = ep(tc.tile_pool(name="qkv_sb", bufs=1))
            wppool = ep(tc.tile_pool(name="wppool", bufs=1))
            dram = ep(tc.tile_pool(name="dram", bufs=1, space="DRAM"))
            # streaming SBUF pools
            xtp = ep(tc.tile_pool(name="xt", bufs=20))
            stgp = ep(tc.tile_pool(name="stg", bufs=4))
            tmpp = ep(tc.tile_pool(name="tmp", bufs=8))
            smlp = ep(tc.tile_pool(name="sml", bufs=8))
            vtp = ep(tc.tile_pool(name="vt", bufs=2))
            ptp = ep(tc.tile_pool(name="pt", bufs=8))
            otp = ep(tc.tile_pool(name="ot", bufs=4))
            rdvp = ep(tc.tile_pool(name="rdv", bufs=4))
            ofp = ep(tc.tile_pool(name="ofp", bufs=8))
            obp = ep(tc.tile_pool(name="obp", bufs=2))
            # PSUM banks: qp 1 + aux 1 + st 2x2 + po 2 = 8
            qpp = ep(tc.tile_pool(name="qp", bufs=1, space="PSUM"))
            auxp = ep(tc.tile_pool(name="aux", bufs=1, space="PSUM"))
            stp = ep(tc.tile_pool(name="stp", bufs=2, space="PSUM"))
            pop = ep(tc.tile_pool(name="pop", bufs=2, space="PSUM"))

            ind2col = consts.tile([128, 2], bf16)
            nc.sync.dma_start(ind2col[:], ind2col_d[:])
            indbc = consts.tile([2, 128], bf16)
            nc.sync.dma_start(indbc[:], indbc_d[:])
            ones64 = consts.tile([1, 64], bf16)
            nc.sync.dma_start(ones64[:], ones64_d[:])
            ident = consts.tile([128, 128], bf16)
            nc.sync.dma_start(ident[:], ident_d[:])
            bp_sb = consts.tile([128, 8], fp32)
            nc.sync.dma_start(bp_sb[:], bp_d[:])

            # tables are loaded inside the emission section, after the
            # first x chunk's DMAs (they are only needed once rope starts)
            qc_sb = tabs.tile([128, L], bf16)
            qs_sb = tabs.tile([128, L], bf16)
            kc_sb = tabs.tile([128, L], bf16)
            ks_sb = tabs.tile([128, L], bf16)

            def load_tables():
                nc.sync.dma_start(qc_sb[:], qc[:])
                nc.sync.dma_start(qs_sb[:], qs[:])
                nc.sync.dma_start(kc_sb[:], kc[:])
                nc.sync.dma_start(ks_sb[:], ks[:])

            wq_sb = []
            for kk in range(8):
                t = wqp.tile([128, F], bf16, name=f"wq{kk}")
                nc.sync.dma_start(t[:], wq[128 * kk:128 * (kk + 1), :])
                wq_sb.append(t)
            wp_sb = [wppool.tile([128, DIM], bf16, name=f"wp{ff}")
                     for ff in range(8)]

            def load_wp():
                for ff in range(8):
                    nc.sync.dma_start(wp_sb[ff][:],
                                      wp_d[128 * ff:128 * (ff + 1), :])

            # per-chunk persistent activations
            qTn = [qkv_sb.tile([128, CH], bf16, name=f"qTn{c}")
                   for c in range(NCH)]
            kTn = [qkv_sb.tile([128, CH], bf16, name=f"kTn{c}")
                   for c in range(NCH)]
            # v per chunk: 4 m-tiles of [128, 130] (64 vA | 1 | 64 vB | 1)
            v_sb = [qkv_sb.tile([128, 4 * 130], bf16, name=f"v{c}")
                    for c in range(NCH)]
            for c in range(NCH):
                nc.gpsimd.memset(v_sb[c][:], 1.0)

            # per-batch A2A buffers: dest core j's block is this core's two
            # heads (128 rows) for j's 256 columns of batch b
            a2a_in = [dram.tile([NC * 128, 256], bf16, name=f"a2a_in{b}")
                      for b in range(2)]
            a2a_out = [dram.tile([NC * 128, 256], bf16, name=f"a2a_out{b}")
                       for b in range(2)]

            def qkv_piece(ch, xt, tix):
                """One tix (q=0 / k=1 / v=2) of a chunk's qkv+rope; ~1.7us
                of dense PE work so it can interleave with attention."""
                lsl = slice(CH * (ch % 4), CH * (ch % 4) + CH)
                ct, stb, dst = [(qc_sb, qs_sb, qTn[ch]),
                                (kc_sb, ks_sb, kTn[ch]),
                                (None, None, None)][tix]
                p = qpp.tile([128, CH], fp32, tag="qp")
                for kk in range(8):
                    nc.tensor.matmul(
                        p[:], wq_sb[kk][:, 128 * tix:128 * (tix + 1)],
                        xt[kk][:], start=(kk == 0), stop=(kk == 7))
                if tix == 2:
                    # v: bf16 copy now; the PE transpose is a separate
                    # piece (v_finish) so it never FIFO-stalls the PE
                    # waiting on this DVE copy
                    vt = vtp.tile([128, CH], bf16, tag="vt",
                                  name=f"vt{ch}")
                    nc.vector.tensor_copy(vt[:], p[:])
                    vt_hold[ch] = vt
                    return
                # rmsnorm: ssq -> 1/sqrt via Exp(-0.5*Ln) -> broadcast
                stgd = stgp.tile([128, CH], bf16, tag="stg")
                nc.vector.tensor_copy(stgd[:], p[:])
                sq = stgp.tile([128, CH], bf16, tag="sq")
                nc.vector.tensor_mul(sq[:], stgd[:], stgd[:])
                ssq = auxp.tile([2, CH], fp32, tag="aux")
                nc.tensor.matmul(ssq[:], ind2col[:], sq[:],
                                 start=True, stop=True)
                lssq = smlp.tile([2, CH], fp32, tag="lssq")
                nc.scalar.activation(lssq[:], ssq[:], AF.Ln)
                ivb = smlp.tile([2, CH], bf16, tag="ivb")
                nc.scalar.activation(ivb[:], lssq[:], AF.Exp, scale=-0.5)
                invb = auxp.tile([128, CH], fp32, tag="aux")
                nc.tensor.matmul(invb[:], indbc[:], ivb[:],
                                 start=True, stop=True)
                invbs = tmpp.tile([128, CH], bf16, tag="invbs")
                nc.vector.tensor_copy(invbs[:], invb[:])
                # rope on the bf16 copy (tables are head-duplicated
                # [128, L] so every mul is partition-aligned)
                tc_ = tmpp.tile([128, CH], bf16, tag="tc")
                nc.vector.tensor_mul(tc_[:], stgd[:], ct[:, lsl])
                ts_ = tmpp.tile([128, CH], bf16, tag="ts")
                for h in range(2):
                    r0 = 64 * h
                    nc.vector.tensor_mul(
                        ts_[r0:r0 + 32, :], stgd[r0 + 32:r0 + 64, :],
                        stb[r0 + 32:r0 + 64, lsl])
                    nc.vector.tensor_mul(
                        ts_[r0 + 32:r0 + 64, :], stgd[r0:r0 + 32, :],
                        stb[r0:r0 + 32, lsl])
                o12 = tmpp.tile([128, CH], bf16, tag="o12")
                nc.vector.tensor_add(o12[:], tc_[:], ts_[:])
                nc.vector.tensor_mul(dst[:, :], o12[:], invbs[:])

            vt_hold = {}

            def v_finish(ch):
                # PE transpose of v, interleaved next to the ones columns
                # ([64 vA | 1 | 64 vB | 1] per m-tile)
                vt = vt_hold.pop(ch)
                tp = qpp.tile([128, CH], bf16, tag="qp")
                for blk in range(4):
                    nc.tensor.transpose(
                        tp[:, 128 * blk:128 * (blk + 1)],
                        vt[:, 128 * blk:128 * (blk + 1)], ident[:])
                for blk in range(4):
                    nc.vector.tensor_copy(
                        v_sb[ch][:, 130 * blk:130 * blk + 64],
                        tp[:, 128 * blk:128 * blk + 64])
                    nc.vector.tensor_copy(
                        v_sb[ch][:, 130 * blk + 65:130 * blk + 129],
                        tp[:, 128 * blk + 64:128 * (blk + 1)])

            def load_xt(ch):
                c0 = CH * ch
                xt = []
                for kk in range(8):
                    t = xtp.tile([128, CH], bf16, tag="xt")
                    nc.sync.dma_start(
                        t[:], xT[128 * kk:128 * (kk + 1), c0:c0 + CH])
                    xt.append(t)
                return xt

            def proj_piece(b, dd, of):
                pr = qpp.tile([128, 256], fp32, tag="qp")
                for ff in range(8):
                    nc.tensor.matmul(
                        pr[:], wp_sb[ff][:, 128 * dd:128 * (dd + 1)],
                        of[ff][:], start=(ff == 0), stop=(ff == 7))
                ob = obp.tile([128, 256], fp32, tag="ob")
                nc.vector.tensor_scalar_add(ob[:], pr[:],
                                            bp_sb[:, dd:dd + 1])
                nc.sync.dma_start(
                    out_d[128 * dd:128 * (dd + 1), 256 * b:256 * b + 256],
                    ob[:])

            def load_of(b):
                of = []
                for ff in range(8):
                    t = ofp.tile([128, 256], bf16, tag="of",
                                 name=f"of{b}_{ff}")
                    nc.sync.dma_start(
                        t[:], a2a_out[b][128 * ff:128 * (ff + 1), :])
                    of.append(t)
                return of

            def attention_batch(b, filler):
                """32 super-iterations (4 sweeps x 8 m-pairs), software-
                pipelined: each super-iter computes scores for two adjacent
                m-tiles per head into a [128, 1024] two-bank PSUM tile and
                runs one FD=1024 exp per head (amortizes the ACT per-
                instruction overhead). o-matmuls lag by one super-iter so
                the in-order PE queue never stalls on a fresh exp; the
                softmax division trails each sweep by 1-3 super-iters.
                `filler` maps super-iter -> thunks emitting independent PE
                work (next batch's qkv, previous batch's proj)."""
                pts_hist = {}
                po_all = {}
                div_q = {}

                def emit_o(u):
                    s, mp = u // 8, 2 * (u % 8)
                    pts = pts_hist.pop(u)
                    for dm in range(2):
                        m = mp + dm
                        cm = 4 * b + m // 4
                        vo = 130 * (m % 4)
                        for h in range(2):
                            nc.tensor.matmul(
                                po_all[s][h][:],
                                v_sb[cm][:, vo + 65 * h:vo + 65 * h + 65],
                                pts[h][:, CH * dm:CH * (dm + 1)],
                                start=(m == 0), stop=(m == 15))

                def div_stage(s, stage):
                    # stage 0: denominators -> reciprocal; 1: broadcast;
                    # 2: divide + stage into the A2A buffer
                    st_ = div_q[s]
                    if stage == 0:
                        po = po_all[s]
                        for h in range(2):
                            rc0 = rdvp.tile([1, CH], fp32, tag="rc0")
                            nc.scalar.activation(rc0[:], po[h][64:65, :],
                                                 AF.Copy)
                            rc = rdvp.tile([1, CH], fp32, tag="rc")
                            nc.vector.reciprocal_approx_fast(rc[:], rc0[:])
                            rcb = rdvp.tile([1, CH], bf16, tag="rcb")
                            nc.vector.tensor_copy(rcb[:], rc[:])
                            st_.append(rcb)
                    elif stage == 1:
                        for h in range(2):
                            rb = stp.tile([64, CH], fp32, tag="st")
                            nc.tensor.matmul(rb[:], ones64[:], st_[h][:],
                                             start=True, stop=True)
                            rbs = rdvp.tile([64, CH], bf16, tag="rbs")
                            nc.vector.tensor_copy(rbs[:], rb[:])
                            st_.append(rbs)
                    else:
                        po = po_all.pop(s)
                        for h in range(2):
                            ot = otp.tile([64, CH], bf16, tag="ot")
                            nc.vector.tensor_mul(ot[:], po[h][0:64, :],
                                                 st_[2 + h][:])
                            for half in range(2):
                                j = 2 * s + half
                                nc.sync.dma_start(
                                    a2a_in[b][128 * j + 64 * h:
                                              128 * j + 64 * h + 64, :],
                                    ot[:, 256 * half:256 * half + 256])
                        del div_q[s]

                for u in range(36):
                    if 1 <= u <= 32:
                        emit_o(u - 1)
                    # division stages trail each sweep by 1/2/3 super-iters
                    # so nothing ever waits on a fresh dependency
                    if u >= 9 and (u - 9) % 8 == 0 and (u - 9) // 8 < 4:
                        s_ = (u - 9) // 8
                        div_q[s_] = []
                        div_stage(s_, 0)
                    if u >= 10 and (u - 10) % 8 == 0 and (u - 10) // 8 < 4:
                        div_stage((u - 10) // 8, 1)
                    if u >= 11 and (u - 11) % 8 == 0 and (u - 11) // 8 < 4:
                        div_stage((u - 11) // 8, 2)
                    if u < 32:
                        s, mp = u // 8, 2 * (u % 8)
                        if mp == 0:
                            po_all[s] = [
                                pop.tile([65, CH], fp32, tag="po",
                                         name=f"po{b}{s}{h}")
                                for h in range(2)]
                        cl = 4 * b + s
                        sts = [stp.tile([128, 2 * CH], fp32, tag="st",
                                        name=f"st{b}_{u}_{h}")
                               for h in range(2)]
                        # dm outer / h inner keeps the two heads' K=64
                        # matmuls adjacent -> concurrent row-tiled pairs
                        for dm in range(2):
                            m = mp + dm
                            cm = 4 * b + m // 4
                            mo = 128 * (m % 4)
                            for h in range(2):
                                hr = slice(64 * h, 64 * h + 64)
                                nc.tensor.matmul(
                                    sts[h][:, CH * dm:CH * (dm + 1)],
                                    kTn[cm][hr, mo:mo + 128],
                                    qTn[cl][hr, :], start=True, stop=True)
                        pts = []
                        for h in range(2):
                            pt = ptp.tile([128, 2 * CH], bf16, tag="pt")
                            nc.scalar.activation(pt[:], sts[h][:], AF.Exp)
                            pts.append(pt)
                        pts_hist[u] = pts
                    for thunk in filler.get(u, []):
                        thunk()

            import concourse.mybir as mybir2

            def emit_a2a(b):
                nc.gpsimd.collective_compute(
                    "AllToAll", mybir2.AluOpType.bypass,
                    replica_groups=[list(range(NC))],
                    ins=[a2a_in[b][:]],
                    outs=[a2a_out[b][:]],
                )

            # ---- emission ----
            # prefix: qkv for b=0 (attention b0 starts as chunk 0 lands);
            # x chunk 0 is the critical first DMA, tables follow it
            for ch in range(4):
                xt = load_xt(ch)
                if ch == 0:
                    load_tables()
                for tix in range(3):
                    qkv_piece(ch, xt, tix)
                v_finish(ch)
            load_wp()  # needed only by proj; keep off the startup DMA path

            # attention b0 with b1's qkv interleaved as PE filler; the
            # last chunk's pieces land on the sweep-3 division drain so
            # the PE never idles long enough for HAM to re-throttle
            # qkv pieces sit on the sweep-boundary division windows
            # (u = 8s+9..), where attention itself leaves the PE sparse
            filler0 = {}
            xt_hold = {}
            for i, ch in enumerate(range(4, 8)):
                base = (4, 12, 20, 28)[i]

                def mk_load(ch=ch):
                    xt_hold[ch] = load_xt(ch)
                filler0.setdefault(base - 1, []).append(mk_load)
                for tix in range(3):
                    def mk(ch=ch, tix=tix):
                        qkv_piece(ch, xt_hold[ch], tix)
                    filler0.setdefault(base + 2 * tix, []).append(mk)

                def mkv(ch=ch):
                    v_finish(ch)
                filler0.setdefault(base + 6, []).append(mkv)
            attention_batch(0, filler0)
            emit_a2a(0)

            # attention b1 with proj(b0) on its division windows
            filler1 = {}
            of_hold = {}

            def mk_of0():
                of_hold[0] = load_of(0)
            filler1.setdefault(5, []).append(mk_of0)
            for dd, pos in enumerate((9, 10, 17, 18, 25, 26, 33, 34)):
                def mkp(dd=dd):
                    proj_piece(0, dd, of_hold[0])
                filler1.setdefault(pos, []).append(mkp)
            attention_batch(1, filler1)
            emit_a2a(1)

            of1 = load_of(1)
            for dd in range(8):
                proj_piece(1, dd, of1)

            if taps:
                nc.sync.dma_start(tap_qtn[:], qTn[0][:])
                nc.sync.dma_start(tap_ktn[:], kTn[0][:])
                nc.sync.dma_start(tap_v[:], v_sb[0][:])
                nc.sync.dma_start(tap_a2ain[:], a2a_in[0][:])
                nc.sync.dma_start(tap_a2aout[:], a2a_out[0][:])

    nc.compile()
    return nc


def _run(inputs, trace=False, trace_kwargs=None):
    from concourse.bass_utils import run_bass_kernel_spmd

    if "nc" not in _CACHE:
        _CACHE["nc"] = _build()
    nc = _CACHE["nc"]
    in_maps = _host_inputs(**inputs)
    res = run_bass_kernel_spmd(
        nc, in_maps, core_ids=list(range(NC)), trace=trace,
        **(trace_kwargs or {}))
    return res


def _gather(res):
    fullT = np.empty((DIM, BL), np.float32)
    for c in range(NC):
        t = res.results[c]["out"]
        fullT[:, 256 * c:256 * (c + 1)] = t[:, 0:256]
        fullT[:, 2048 + 256 * c:2048 + 256 * (c + 1)] = t[:, 256:512]
    return np.ascontiguousarray(fullT.T).reshape(B, L, DIM).astype(np.float32)


def kernel(x, Wqkv, q_scale, k_scale, Wproj, bproj):
    res = _run(dict(x=x, Wqkv=Wqkv, q_scale=q_scale, k_scale=k_scale,
                    Wproj=Wproj, bproj=bproj))
    return _gather(res)


if __name__ == "__main__":
    rng = np.random.default_rng(0)
    x = rng.standard_normal((B, L, DIM), dtype=np.float32)
    Wqkv_ = rng.standard_normal((DIM, 3 * DIM), dtype=np.float32) * DIM ** -0.5
    Wproj_ = rng.standard_normal((DIM, DIM), dtype=np.float32) * DIM ** -0.5
    out = kernel(x=x, Wqkv=Wqkv_, q_scale=np.ones(HD, np.float32),
                 k_scale=np.ones(HD, np.float32), Wproj=Wproj_,
                 bproj=np.zeros(DIM, np.float32))
    print(out.shape, out.dtype)
